# revision 25
# baseline (speedup 1.0000x reference)
"""Multi-head causal attention (with faithful reference bugs) on 8 TRN2 cores.

Reference semantics (B=4, T=2048, D=1024, H=16, hd=64):
    q = (x @ Wq.T) viewed (B,T,H,hd) -> (B,H,T,hd); same k, v
    scores = (q @ k.T) * sqrt(D)            # "bug": / D**-0.5
    causal mask, softmax
    out = attn @ v                          # (B,H,T,hd)
    att = out.reshape(B, T, H*hd)           # "bug": no transpose back
    y = att @ Wo.T

Because of the reshape bug, output rows group by head: rows
[128h, 128h+128) of y[b] depend only on head h.  Sharding: 8 cores =
(batch b, head-group g); each core computes y[b, 1024g:1024g+1024, :]
independently (no collectives).

v3 design: heads processed in PAIRS packed onto the PE array.
  - q/k live in paired tiles [128, T]: rows 0-63 = head 2p, 64-127 =
    head 2p+1 (fp16, q pre-scaled by 32).  No DRAM spill.
  - each A/B pair of score matmuls (K=64) targets ONE fused [128,1024]
    PSUM tile ([A | B] halves) at tile_position (0,0)/(64,0): both MMs
    become ready on the same buffer event and run CONCURRENTLY on the
    two halves of the array.
  - prepass (row max): packed q.k chunks + ONE fused DVE reduce per
    chunk ([128, 2, cw] -> [128, 2]) so the tile frees on one event.
  - main: packed q.k + the -m subtraction as K=1 rank-1 matmuls on
    32-row strips + -1e9*I causal mask matmuls; exp -> fp16 w~;
    AV via v^T (ones column emits the denominator row).
  - normalization: denominator row bounced [1,512]->[128,4] via
    SBUF->SBUF DMA so DVE reciprocal runs partition-major; GPSIMD
    broadcast; reshape-bug scatter-multiply on GPSIMD (keeps DVE free
    for the prepass reductions).
  - PSUM: shared score pool (3 x [128,1024] = 6 banks) for prepass
    chunks AND main s~ tiles + 2 AV banks = 8.
"""

import numpy as np
import ml_dtypes

B, T, D, H = 4, 2048, 1024, 16
HD = D // H  # 64
HL = H // 2  # heads per core = 8
NP = HL // 2  # head pairs per core = 4
SCALE = float(np.sqrt(D))  # 32.0
NEG = -1.0e9

_CACHE = {}


def _build():
    import concourse.bacc as bacc
    import concourse.mybir as mybir
    import concourse.tile as tile
    from concourse.masks import make_identity

    dt = mybir.dt
    f32, f16, bf16 = dt.float32, dt.float16, dt.bfloat16
    Exp = mybir.ActivationFunctionType.Exp
    AX = mybir.AxisListType.X

    nc = bacc.Bacc("TRN2", target_bir_lowering=False, debug=False, num_devices=8)

    # ---- DRAM I/O ----
    xT = nc.dram_tensor("xT", [D, T], f16, kind="ExternalInput")  # x[b].T
    wqT = nc.dram_tensor("wqT", [D, 512], f16, kind="ExternalInput")  # Wq[g].T
    wkT = nc.dram_tensor("wkT", [D, 512], f16, kind="ExternalInput")
    wvT = nc.dram_tensor("wvT", [D, 512], f16, kind="ExternalInput")
    woT = nc.dram_tensor("woT", [D, D], f16, kind="ExternalInput")  # Wo.T fp16
    # 0/1 step masks (c<p), (c>p) and -1e9*I for PE-side causal masking
    vmaskD = nc.dram_tensor("vmaskD", [128, 128], bf16, kind="ExternalInput")
    vmaskP = nc.dram_tensor("vmaskP", [128, 128], bf16, kind="ExternalInput")
    negI = nc.dram_tensor("negI", [128, 128], bf16, kind="ExternalInput")
    out = nc.dram_tensor("out", [1024, D], f32, kind="ExternalOutput")
    # scratch rows for the DMA-broadcast of 1/denominator (one per qb/head)
    drow = nc.dram_tensor("drow", [32, 512], f32)

    with tile.TileContext(nc) as tc:
        with (
            tc.tile_pool(name="const", bufs=1) as cpool,
            tc.tile_pool(name="vres", bufs=1) as vpool,
            tc.tile_pool(name="qk", bufs=4) as qkpool,
            tc.tile_pool(name="stat", bufs=4) as mpool,
            tc.tile_pool(name="mhatp", bufs=2) as mhpool,
            tc.tile_pool(name="wexp", bufs=6) as wpoolx,
            tc.tile_pool(name="tt", bufs=4) as tpool,
            tc.tile_pool(name="sm", bufs=6) as smpool,
            tc.tile_pool(name="nrm", bufs=4) as npool,
            tc.tile_pool(name="fsp", bufs=3) as fpool,
            # fused A|B score tiles: prepass chunks AND main s~ tiles
            tc.tile_pool(name="score_ps", bufs=3, space="PSUM") as scpool,
        ):
            # ---- constants / resident tensors ----
            vmaskD_t = cpool.tile([128, 128], bf16)
            nc.sync.dma_start(vmaskD_t[:], vmaskD[:])
            vmaskP_t = cpool.tile([128, 128], bf16)
            nc.sync.dma_start(vmaskP_t[:], vmaskP[:])
            negI_t = cpool.tile([128, 128], bf16)
            nc.sync.dma_start(negI_t[:], negI[:])
            ident = cpool.tile([128, 128], f32)
            make_identity(nc, ident[:])
            ones_t = cpool.tile([128, 128], f16)
            nc.gpsimd.memset(ones_t[:], 1.0)
            wo_sb = cpool.tile([128, 8, 1024], f16)
            nc.sync.dma_start(wo_sb[:], woT.rearrange("(a p) m -> p a m", p=128))
            # v resident: [p, ttile, head, 65] fp16, col 64 = ones
            v_sb = vpool.tile([128, 16, HL, 65], f16)
            nc.gpsimd.memset(v_sb[:, :, :, 64:65], 1.0)

            qk_tiles = {}  # pair -> (qpair, kpair), rows 0-63 = head 2p
            mhat_tiles = {}  # pair -> [128, T] f16, -m at rows {0,32}=A {64,96}=B

            def gen_prepass_pair(p):
                qp, kp = qk_tiles[p]
                mhA = mpool.tile([128, 16], f32, tag="mh", name=f"mhA{p}")
                mhB = mpool.tile([128, 16], f32, tag="mh", name=f"mhB{p}")
                for qi in range(16):
                    kext = 128 * (qi + 1)
                    nchk = (kext + 511) // 512
                    cm = mpool.tile([128, 8], f32, tag="cmx", name="cm")
                    for ch in range(nchk):
                        c0 = ch * 512
                        cw = min(512, kext - c0)
                        last = ch == nchk - 1
                        pr = scpool.tile([128, 1024], f32, tag="sc", name="pr")
                        nc.tensor.matmul(
                            pr[:, 0:cw],
                            qp[0:64, qi * 128 : (qi + 1) * 128],
                            kp[0:64, c0 : c0 + cw],
                            start=True,
                            stop=not last,
                            skip_group_check=True,
                            tile_position=(0, 0),
                        )
                        nc.tensor.matmul(
                            pr[:, 512 : 512 + cw],
                            qp[64:128, qi * 128 : (qi + 1) * 128],
                            kp[64:128, c0 : c0 + cw],
                            start=True,
                            stop=not last,
                            skip_group_check=True,
                            tile_position=(64, 0),
                        )
                        if last:
                            # diagonal causal mask accumulated on the PE
                            nc.tensor.matmul(
                                pr[:, cw - 128 : cw],
                                negI_t[:],
                                vmaskP_t[:],
                                start=False,
                                stop=True,
                                skip_group_check=True,
                            )
                            nc.tensor.matmul(
                                pr[:, 384 + cw : 512 + cw],
                                negI_t[:],
                                vmaskP_t[:],
                                start=False,
                                stop=True,
                                skip_group_check=True,
                            )
                        # fused per-head chunk max: [128, 2, cw] -> [128, 2]
                        nc.vector.reduce_max(
                            cm[:, 2 * ch : 2 * ch + 2],
                            pr[:].rearrange("q (h f) -> q h f", h=2)[:, :, 0:cw],
                            axis=AX,
                        )
                        yield
                    for hh, mh in ((0, mhA), (1, mhB)):
                        if nchk == 1:
                            nc.vector.tensor_scalar_mul(
                                mh[:, qi : qi + 1], cm[:, hh : hh + 1], -1.0
                            )
                        else:
                            nc.vector.reduce_max(
                                mh[:, qi : qi + 1],
                                cm[:, hh : 2 * nchk : 2],
                                axis=AX,
                                negate=True,
                            )
                # -m transposed to q-major rows, replicated at 32-row strips
                mha = mhpool.tile([128, T], f16, tag="mhat", name=f"mhat{p}")
                mhat_tiles[p] = mha
                for mh, rows in ((mhA, (0, 32)), (mhB, (64, 96))):
                    mt_ps = scpool.tile([16, 128], f32, tag="sc", name="mt_ps")
                    nc.tensor.transpose(mt_ps[:], mh[:], ident[:])
                    mts = mpool.tile([16, 128], f16, tag="mts", name="mts")
                    nc.scalar.copy(mts[:], mt_ps[:])
                    for r in rows:
                        nc.sync.dma_start(mha[r : r + 1, :], mts[:])
                    yield

            # ================= Phase 1: projections =================
            with (
                tc.tile_pool(name="wgt", bufs=1) as wpool,
                tc.tile_pool(name="xch", bufs=32) as xpool,
                tc.tile_pool(name="proj_ps", bufs=2, space="PSUM") as ppool,
            ):
                wq_sb = wpool.tile([128, 8, 512], f16, tag="w")
                wk_sb = wpool.tile([128, 8, 512], f16, tag="wk")
                wv_sb = wpool.tile([128, 8, 512], f16, tag="wv")
                # issue weight loads from the ACT queue so they don't
                # serialize behind the x-chunk loads at startup; split each
                # into dc-halves so the first projection MMs start sooner
                wqR = wqT.rearrange("(a p) m -> p a m", p=128)
                wkR = wkT.rearrange("(a p) m -> p a m", p=128)
                wvR = wvT.rearrange("(a p) m -> p a m", p=128)
                for w_dst, w_src in ((wq_sb, wqR), (wk_sb, wkR), (wv_sb, wvR)):
                    nc.scalar.dma_start(w_dst[:, 0:2], w_src[:, 0:2])
                for w_dst, w_src in ((wq_sb, wqR), (wk_sb, wkR), (wv_sb, wvR)):
                    nc.scalar.dma_start(w_dst[:, 2:8], w_src[:, 2:8])

                xTr = xT.rearrange("(a p) m -> p a m", p=128)
                xch = {}
                for tb in range(4):
                    for dc in range(8):
                        xc = xpool.tile([128, 512], f16, tag="x", name=f"x{dc}_{tb}")
                        # split across two DMA queues to halve the startup
                        # critical path
                        eng = nc.gpsimd if dc % 2 == 0 else nc.sync
                        eng.dma_start(xc[:], xTr[:, dc, tb * 512 : (tb + 1) * 512])
                        xch[(dc, tb)] = xc

                for p in range(NP):
                    qpt = qkpool.tile([128, T], f16, tag="qp", name=f"qp{p}")
                    kpt = qkpool.tile([128, T], f16, tag="kp", name=f"kp{p}")
                    qk_tiles[p] = (qpt, kpt)

                for ot in range(4):
                    for wi, (w_sb, scl) in enumerate(
                        ((wq_sb, SCALE), (wk_sb, 1.0))
                    ):
                        for tb in range(4):
                            ps = ppool.tile([128, 512], f32, tag="ps")
                            for dc in range(8):
                                nc.tensor.matmul(
                                    ps[:],
                                    w_sb[:, dc, ot * 128 : (ot + 1) * 128],
                                    xch[(dc, tb)][:],
                                    start=(dc == 0),
                                    stop=(dc == 7),
                                )
                            dst = qk_tiles[ot][wi][:, tb * 512 : (tb + 1) * 512]
                            if scl == 1.0:
                                nc.scalar.copy(dst, ps[:])
                            else:
                                nc.scalar.mul(dst, ps[:], scl)
                    # v projection for this otile's t-range
                    for tt in range(4):
                        ttile = ot * 4 + tt
                        tb, tsub = ttile // 4, ttile % 4
                        ps = ppool.tile([128, 512], f32, tag="ps")
                        for dc in range(8):
                            nc.tensor.matmul(
                                ps[:],
                                xch[(dc, tb)][:, tsub * 128 : (tsub + 1) * 128],
                                wv_sb[:, dc, :],
                                start=(dc == 0),
                                stop=(dc == 7),
                            )
                        nc.scalar.copy(v_sb[:, ttile, :, 0:64], ps[:])
                    if ot == 0:
                        g0 = gen_prepass_pair(0)
                        while next(g0, "end") != "end":
                            pass

            # ============== Phase 2: attention mains ==============
            with tc.tile_pool(name="av_ps", bufs=2, space="PSUM") as avpool:
                th_map = {}

                def gen_main_pair(p):
                    qp, kp = qk_tiles.pop(p)
                    mha = mhat_tiles.pop(p)
                    hA, hB = 2 * p, 2 * p + 1
                    thA = tpool.tile([128, 1024], f16, tag="th", name=f"thA{p}")
                    thB = tpool.tile([128, 1024], f16, tag="th", name=f"thB{p}")
                    th_map[hA], th_map[hB] = thA, thB
                    for qb in range(4):
                        avA = avpool.tile([128, 512], f32, tag="av", name="avA")
                        avB = avpool.tile([128, 512], f32, tag="av", name="avB")
                        nkb = 4 * (qb + 1)
                        pend_av = None
                        for kb in range(nkb):
                            j = kb - 4 * qb
                            t0 = 128 * j if j > 0 else 0
                            diag = j >= 0
                            # fused [A | B] score tile for this k-tile
                            sg = scpool.tile([128, 1024], f32, tag="sc", name="sg")
                            nc.tensor.matmul(
                                sg[:, t0:512],
                                kp[0:64, kb * 128 : (kb + 1) * 128],
                                qp[0:64, qb * 512 + t0 : (qb + 1) * 512],
                                start=True,
                                stop=False,
                                skip_group_check=True,
                                tile_position=(0, 0),
                            )
                            nc.tensor.matmul(
                                sg[:, 512 + t0 : 1024],
                                kp[64:128, kb * 128 : (kb + 1) * 128],
                                qp[64:128, qb * 512 + t0 : (qb + 1) * 512],
                                start=True,
                                stop=False,
                                skip_group_check=True,
                                tile_position=(64, 0),
                            )
                            # -m rank-1 updates on 32-row strips (A/B packed;
                            # kb parity alternates strips so consecutive
                            # k-tiles can overlap too)
                            rA = 32 * (kb % 2)
                            rB = 64 + 32 * (kb % 2)
                            nc.tensor.matmul(
                                sg[:, t0:512],
                                ones_t[rA : rA + 1, :],
                                mha[rA : rA + 1, qb * 512 + t0 : (qb + 1) * 512],
                                start=False,
                                stop=not diag,
                                skip_group_check=True,
                                tile_position=(rA, 0),
                            )
                            nc.tensor.matmul(
                                sg[:, 512 + t0 : 1024],
                                ones_t[rB : rB + 1, :],
                                mha[rB : rB + 1, qb * 512 + t0 : (qb + 1) * 512],
                                start=False,
                                stop=not diag,
                                skip_group_check=True,
                                tile_position=(rB, 0),
                            )
                            # diagonal causal masks
                            if diag:
                                nc.tensor.matmul(
                                    sg[:, t0 : t0 + 128],
                                    negI_t[:],
                                    vmaskD_t[:],
                                    start=False,
                                    stop=True,
                                    skip_group_check=True,
                                )
                                nc.tensor.matmul(
                                    sg[:, 512 + t0 : 512 + t0 + 128],
                                    negI_t[:],
                                    vmaskD_t[:],
                                    start=False,
                                    stop=True,
                                    skip_group_check=True,
                                )
                            # exp -> fp16 w~ ([A | B]); one call covering
                            # [t0:1024] — the stale middle [512,512+t0) of
                            # diag tiles is exp'd but never read downstream
                            wsb = wpoolx.tile([128, 1024], f16, tag="wsb", name="wsb")
                            nc.scalar.activation(wsb[:, t0:1024], sg[:, t0:1024], Exp)

                            # AV software-pipelined by one k-tile: emit the
                            # PREVIOUS kb's AV (its exp surely done) so the PE
                            # never idles waiting on this kb's exp
                            def emit_av(wsb_p, kb_p, t0_p):
                                nc.tensor.matmul(
                                    avA[0:65, t0_p:512],
                                    v_sb[:, kb_p, hA, :],
                                    wsb_p[:, t0_p:512],
                                    start=(kb_p == 0),
                                    stop=(kb_p == nkb - 1),
                                    skip_group_check=True,
                                )
                                nc.tensor.matmul(
                                    avB[0:65, t0_p:512],
                                    v_sb[:, kb_p, hB, :],
                                    wsb_p[:, 512 + t0_p : 1024],
                                    start=(kb_p == 0),
                                    stop=(kb_p == nkb - 1),
                                    skip_group_check=True,
                                )

                            if pend_av is not None:
                                emit_av(*pend_av)
                            pend_av = (wsb, kb, t0)
                            yield
                        if pend_av is not None:
                            emit_av(*pend_av)
                            pend_av = None
                        # drain AV psum; normalize; scatter (reshape bug)
                        for hh, (av, th) in enumerate(((avA, thA), (avB, thB))):
                            avs = smpool.tile([65, 512], f32, tag="avs", name="avs")
                            nc.scalar.copy(avs[:], av[0:65, :])
                            # denominator row -> partition-major for cheap DVE
                            # reciprocal, then back to a row for the broadcast
                            rpm = npool.tile([128, 4], f32, tag="rpm", name="rpm")
                            nc.sync.dma_start(rpm[:], avs[64:65, :])
                            rpc = npool.tile([128, 4], f32, tag="rpc", name="rpc")
                            nc.vector.reciprocal(rpc[:], rpm[:])
                            # broadcast to 64 partitions via a DRAM-bounce DMA
                            # (keeps GPSIMD on a single ucode lib: tensor_mul)
                            di = (p * 4 + qb) * 2 + hh
                            nc.sync.dma_start(drow[di : di + 1, :], rpc[:])
                            bc = npool.tile([64, 512], f32, tag="bc", name="bc")
                            nc.sync.dma_start(
                                bc[:], drow[di : di + 1, :].to_broadcast([64, 512])
                            )
                            for jpar in range(2):
                                src = avs[0:64, jpar:512:2].rearrange(
                                    "p (i jj) -> p jj i", jj=8
                                )
                                scl = bc[:, jpar:512:2].rearrange(
                                    "p (i jj) -> p jj i", jj=8
                                )
                                dst = th[
                                    jpar * 64 : jpar * 64 + 64, :
                                ].rearrange("p (jj i4) -> p jj i4", jj=8)[
                                    :, :, qb * 32 : (qb + 1) * 32
                                ]
                                nc.gpsimd.tensor_mul(dst, src, scl)
                            yield

                def gen_wo(h):
                    # output projection, deferred to overlap the next pair's
                    # attention units.  fo lives in the score pool (NOT the
                    # AV pool) so it never blocks the next qb's accumulators.
                    th = th_map.pop(h)
                    for cb in range(2):
                        fo = scpool.tile([128, 512], f32, tag="sc", name="fo")
                        for jj in range(8):
                            nc.tensor.matmul(
                                fo[:],
                                th[:, jj * 128 : (jj + 1) * 128],
                                wo_sb[:, jj, cb * 512 : (cb + 1) * 512],
                                start=(jj == 0),
                                stop=(jj == 7),
                            )
                            if jj == 3:
                                yield
                        fs = fpool.tile([128, 512], f32, tag="fs", name="fs")
                        nc.vector.tensor_copy(fs[:], fo[:])
                        nc.sync.dma_start(
                            out[h * 128 : (h + 1) * 128, cb * 512 : (cb + 1) * 512],
                            fs[:],
                        )
                        yield

                # interleave: main(p) units with prepass(p+1) chunks and
                # Wo(p-1) chunks so all engines stay fed
                pending_wo = []
                for p in range(NP):
                    gm = gen_main_pair(p)
                    gp = gen_prepass_pair(p + 1) if p + 1 < NP else None
                    done_m = False
                    done_p = gp is None
                    step = 0
                    while not done_m or not done_p:
                        # wo first: its MMs are always-ready PE filler that
                        # plugs micro-gaps (keeps HAM at full clock)
                        if pending_wo:
                            if next(pending_wo[0], "end") == "end":
                                pending_wo.pop(0)
                        if not done_m:
                            done_m = next(gm, "end") == "end"
                            if done_m:
                                # release wo immediately — don't serialize it
                                # behind the prepass tail
                                pending_wo.append(gen_wo(2 * p))
                                pending_wo.append(gen_wo(2 * p + 1))
                        if not done_p:
                            done_p = next(gp, "end") == "end"
                        # prepass has ~1.5x the units of main: pump extra so
                        # the pair-boundary barrier (mhat) has no tail
                        if not done_p and step % 2 == 0:
                            done_p = next(gp, "end") == "end"
                        step += 1
                for g in pending_wo:
                    while next(g, "end") != "end":
                        pass
    nc.compile()
    return nc


def _consts():
    p = np.arange(128)[:, None]
    c = np.arange(128)[None, :]
    vmaskD = (c < p).astype(ml_dtypes.bfloat16)
    vmaskP = (c > p).astype(ml_dtypes.bfloat16)
    negI = (NEG * np.eye(128)).astype(ml_dtypes.bfloat16)
    return vmaskD, vmaskP, negI


def kernel(x, Wq, Wk, Wv, Wo):
    x = np.asarray(x, dtype=np.float32)
    Wq = np.asarray(Wq, dtype=np.float32)
    Wk = np.asarray(Wk, dtype=np.float32)
    Wv = np.asarray(Wv, dtype=np.float32)
    Wo = np.asarray(Wo, dtype=np.float32)

    if "nc" not in _CACHE:
        _CACHE["nc"] = _build()
    nc = _CACHE["nc"]

    from concourse.bass_utils import run_bass_kernel_spmd

    vmaskD, vmaskP, negI = _consts()
    woT = np.ascontiguousarray(Wo.T).astype(np.float16)
    in_maps = []
    for c in range(8):
        b, g = c // 2, c % 2
        sl = slice(512 * g, 512 * (g + 1))
        in_maps.append(
            {
                "xT": np.ascontiguousarray(x[b].T).astype(np.float16),
                "wqT": np.ascontiguousarray(Wq[sl, :].T).astype(np.float16),
                "wkT": np.ascontiguousarray(Wk[sl, :].T).astype(np.float16),
                "wvT": np.ascontiguousarray(Wv[sl, :].T).astype(np.float16),
                "woT": woT,
                "vmaskD": vmaskD,
                "vmaskP": vmaskP,
                "negI": negI,
            }
        )
    res = run_bass_kernel_spmd(nc, in_maps, list(range(8)))
    _CACHE["last_result"] = res
    y = np.empty((B, T, D), dtype=np.float32)
    for c in range(8):
        b, g = c // 2, c % 2
        y[b, 1024 * g : 1024 * (g + 1), :] = res.results[c]["out"]
    return y


# revision 26
# speedup vs baseline: 1.0090x; 1.0090x over previous
"""Multi-head causal attention (with faithful reference bugs) on 8 TRN2 cores.

Reference semantics (B=4, T=2048, D=1024, H=16, hd=64):
    q = (x @ Wq.T) viewed (B,T,H,hd) -> (B,H,T,hd); same k, v
    scores = (q @ k.T) * sqrt(D)            # "bug": / D**-0.5
    causal mask, softmax
    out = attn @ v                          # (B,H,T,hd)
    att = out.reshape(B, T, H*hd)           # "bug": no transpose back
    y = att @ Wo.T

Because of the reshape bug, output rows group by head: rows
[128h, 128h+128) of y[b] depend only on head h.  Sharding: 8 cores =
(batch b, head-group g); each core computes y[b, 1024g:1024g+1024, :]
independently (no collectives).

v3 design: heads processed in PAIRS packed onto the PE array.
  - q/k live in paired tiles [128, T]: rows 0-63 = head 2p, 64-127 =
    head 2p+1 (fp16, q pre-scaled by 32).  No DRAM spill.
  - each A/B pair of score matmuls (K=64) targets ONE fused [128,1024]
    PSUM tile ([A | B] halves) at tile_position (0,0)/(64,0): both MMs
    become ready on the same buffer event and run CONCURRENTLY on the
    two halves of the array.
  - prepass (row max): packed q.k chunks + ONE fused DVE reduce per
    chunk ([128, 2, cw] -> [128, 2]) so the tile frees on one event.
  - main: packed q.k + the -m subtraction as K=1 rank-1 matmuls on
    32-row strips + -1e9*I causal mask matmuls; exp -> fp16 w~;
    AV via v^T (ones column emits the denominator row).
  - normalization: denominator row bounced [1,512]->[128,4] via
    SBUF->SBUF DMA so DVE reciprocal runs partition-major; GPSIMD
    broadcast; reshape-bug scatter-multiply on GPSIMD (keeps DVE free
    for the prepass reductions).
  - PSUM: shared score pool (3 x [128,1024] = 6 banks) for prepass
    chunks AND main s~ tiles + 2 AV banks = 8.
"""

import numpy as np
import ml_dtypes

B, T, D, H = 4, 2048, 1024, 16
HD = D // H  # 64
HL = H // 2  # heads per core = 8
NP = HL // 2  # head pairs per core = 4
SCALE = float(np.sqrt(D))  # 32.0
NEG = -1.0e9

_CACHE = {}


def _build():
    import concourse.bacc as bacc
    import concourse.mybir as mybir
    import concourse.tile as tile
    from concourse.masks import make_identity

    dt = mybir.dt
    f32, f16, bf16 = dt.float32, dt.float16, dt.bfloat16
    Exp = mybir.ActivationFunctionType.Exp
    AX = mybir.AxisListType.X

    nc = bacc.Bacc("TRN2", target_bir_lowering=False, debug=False, num_devices=8)

    # ---- DRAM I/O ----
    xT = nc.dram_tensor("xT", [D, T], f16, kind="ExternalInput")  # x[b].T
    wqT = nc.dram_tensor("wqT", [D, 512], f16, kind="ExternalInput")  # Wq[g].T
    wkT = nc.dram_tensor("wkT", [D, 512], f16, kind="ExternalInput")
    wvT = nc.dram_tensor("wvT", [D, 512], f16, kind="ExternalInput")
    woT = nc.dram_tensor("woT", [D, D], f16, kind="ExternalInput")  # Wo.T fp16
    # 0/1 step masks (c<p), (c>p) and -1e9*I for PE-side causal masking
    vmaskD = nc.dram_tensor("vmaskD", [128, 128], bf16, kind="ExternalInput")
    vmaskP = nc.dram_tensor("vmaskP", [128, 128], bf16, kind="ExternalInput")
    negI = nc.dram_tensor("negI", [128, 128], bf16, kind="ExternalInput")
    out = nc.dram_tensor("out", [1024, D], f32, kind="ExternalOutput")
    # scratch rows for the DMA-broadcast of 1/denominator (one per qb/head)
    drow = nc.dram_tensor("drow", [32, 512], f32)

    with tile.TileContext(nc) as tc:
        with (
            tc.tile_pool(name="const", bufs=1) as cpool,
            tc.tile_pool(name="vres", bufs=1) as vpool,
            tc.tile_pool(name="qk", bufs=4) as qkpool,
            tc.tile_pool(name="stat", bufs=4) as mpool,
            tc.tile_pool(name="mhatp", bufs=2) as mhpool,
            tc.tile_pool(name="wexp", bufs=6) as wpoolx,
            tc.tile_pool(name="tt", bufs=4) as tpool,
            tc.tile_pool(name="sm", bufs=6) as smpool,
            tc.tile_pool(name="nrm", bufs=4) as npool,
            tc.tile_pool(name="fsp", bufs=3) as fpool,
            # fused A|B score tiles: prepass chunks AND main s~ tiles
            tc.tile_pool(name="score_ps", bufs=3, space="PSUM") as scpool,
        ):
            # ---- constants / resident tensors ----
            vmaskD_t = cpool.tile([128, 128], bf16)
            nc.sync.dma_start(vmaskD_t[:], vmaskD[:])
            vmaskP_t = cpool.tile([128, 128], bf16)
            nc.sync.dma_start(vmaskP_t[:], vmaskP[:])
            negI_t = cpool.tile([128, 128], bf16)
            nc.sync.dma_start(negI_t[:], negI[:])
            ident = cpool.tile([128, 128], f32)
            make_identity(nc, ident[:])
            ones_t = cpool.tile([128, 128], f16)
            nc.gpsimd.memset(ones_t[:], 1.0)
            wo_sb = cpool.tile([128, 8, 1024], f16)
            nc.sync.dma_start(wo_sb[:], woT.rearrange("(a p) m -> p a m", p=128))
            # v resident: [p, ttile, head, 65] fp16, col 64 = ones
            v_sb = vpool.tile([128, 16, HL, 65], f16)
            nc.gpsimd.memset(v_sb[:, :, :, 64:65], 1.0)

            qk_tiles = {}  # pair -> (qpair, kpair), rows 0-63 = head 2p
            mhat_tiles = {}  # pair -> [128, T] f16, -m at rows {0,32}=A {64,96}=B

            def gen_prepass_pair(p):
                qp, kp = qk_tiles[p]
                mhA = mpool.tile([128, 16], f32, tag="mh", name=f"mhA{p}")
                mhB = mpool.tile([128, 16], f32, tag="mh", name=f"mhB{p}")
                for qi in range(16):
                    kext = 128 * (qi + 1)
                    nchk = (kext + 511) // 512
                    cm = mpool.tile([128, 8], f32, tag="cmx", name="cm")
                    for ch in range(nchk):
                        c0 = ch * 512
                        cw = min(512, kext - c0)
                        last = ch == nchk - 1
                        pr = scpool.tile([128, 1024], f32, tag="sc", name="pr")
                        nc.tensor.matmul(
                            pr[:, 0:cw],
                            qp[0:64, qi * 128 : (qi + 1) * 128],
                            kp[0:64, c0 : c0 + cw],
                            start=True,
                            stop=not last,
                            skip_group_check=True,
                            tile_position=(0, 0),
                        )
                        nc.tensor.matmul(
                            pr[:, 512 : 512 + cw],
                            qp[64:128, qi * 128 : (qi + 1) * 128],
                            kp[64:128, c0 : c0 + cw],
                            start=True,
                            stop=not last,
                            skip_group_check=True,
                            tile_position=(64, 0),
                        )
                        if last:
                            # diagonal causal mask accumulated on the PE
                            nc.tensor.matmul(
                                pr[:, cw - 128 : cw],
                                negI_t[:],
                                vmaskP_t[:],
                                start=False,
                                stop=True,
                                skip_group_check=True,
                            )
                            nc.tensor.matmul(
                                pr[:, 384 + cw : 512 + cw],
                                negI_t[:],
                                vmaskP_t[:],
                                start=False,
                                stop=True,
                                skip_group_check=True,
                            )
                        # fused per-head chunk max: [128, 2, cw] -> [128, 2]
                        nc.vector.reduce_max(
                            cm[:, 2 * ch : 2 * ch + 2],
                            pr[:].rearrange("q (h f) -> q h f", h=2)[:, :, 0:cw],
                            axis=AX,
                        )
                        yield
                    for hh, mh in ((0, mhA), (1, mhB)):
                        if nchk == 1:
                            nc.vector.tensor_scalar_mul(
                                mh[:, qi : qi + 1], cm[:, hh : hh + 1], -1.0
                            )
                        else:
                            nc.vector.reduce_max(
                                mh[:, qi : qi + 1],
                                cm[:, hh : 2 * nchk : 2],
                                axis=AX,
                                negate=True,
                            )
                # -m transposed to q-major rows, replicated at 32-row strips
                mha = mhpool.tile([128, T], f16, tag="mhat", name=f"mhat{p}")
                mhat_tiles[p] = mha
                for mh, rows in ((mhA, (0, 32)), (mhB, (64, 96))):
                    mt_ps = scpool.tile([16, 128], f32, tag="sc", name="mt_ps")
                    nc.tensor.transpose(mt_ps[:], mh[:], ident[:])
                    mts = mpool.tile([16, 128], f16, tag="mts", name="mts")
                    nc.scalar.copy(mts[:], mt_ps[:])
                    for r in rows:
                        nc.sync.dma_start(mha[r : r + 1, :], mts[:])
                    yield

            # ================= Phase 1: projections =================
            with (
                tc.tile_pool(name="wgt", bufs=1) as wpool,
                tc.tile_pool(name="xch", bufs=32) as xpool,
                tc.tile_pool(name="proj_ps", bufs=2, space="PSUM") as ppool,
            ):
                wq_sb = wpool.tile([128, 8, 512], f16, tag="w")
                wk_sb = wpool.tile([128, 8, 512], f16, tag="wk")
                wv_sb = wpool.tile([128, 8, 512], f16, tag="wv")
                # issue weight loads from the ACT queue so they don't
                # serialize behind the x-chunk loads at startup; split each
                # into dc-halves so the first projection MMs start sooner
                wqR = wqT.rearrange("(a p) m -> p a m", p=128)
                wkR = wkT.rearrange("(a p) m -> p a m", p=128)
                wvR = wvT.rearrange("(a p) m -> p a m", p=128)
                for w_dst, w_src in ((wq_sb, wqR), (wk_sb, wkR), (wv_sb, wvR)):
                    nc.scalar.dma_start(w_dst[:, 0:2], w_src[:, 0:2])
                for w_dst, w_src in ((wq_sb, wqR), (wk_sb, wkR), (wv_sb, wvR)):
                    nc.scalar.dma_start(w_dst[:, 2:8], w_src[:, 2:8])

                xTr = xT.rearrange("(a p) m -> p a m", p=128)
                xch = {}
                for tb in range(4):
                    for dc in range(8):
                        xc = xpool.tile([128, 512], f16, tag="x", name=f"x{dc}_{tb}")
                        # split across two DMA queues to halve the startup
                        # critical path
                        eng = nc.gpsimd if dc % 2 == 0 else nc.sync
                        eng.dma_start(xc[:], xTr[:, dc, tb * 512 : (tb + 1) * 512])
                        xch[(dc, tb)] = xc

                for p in range(NP):
                    qpt = qkpool.tile([128, T], f16, tag="qp", name=f"qp{p}")
                    kpt = qkpool.tile([128, T], f16, tag="kp", name=f"kp{p}")
                    qk_tiles[p] = (qpt, kpt)

                for ot in range(4):
                    for wi, (w_sb, scl) in enumerate(
                        ((wq_sb, SCALE), (wk_sb, 1.0))
                    ):
                        for tb in range(4):
                            ps = ppool.tile([128, 512], f32, tag="ps")
                            for dc in range(8):
                                nc.tensor.matmul(
                                    ps[:],
                                    w_sb[:, dc, ot * 128 : (ot + 1) * 128],
                                    xch[(dc, tb)][:],
                                    start=(dc == 0),
                                    stop=(dc == 7),
                                )
                            dst = qk_tiles[ot][wi][:, tb * 512 : (tb + 1) * 512]
                            if scl == 1.0:
                                nc.scalar.copy(dst, ps[:])
                            else:
                                nc.scalar.mul(dst, ps[:], scl)
                    # v projection for this otile's t-range
                    for tt in range(4):
                        ttile = ot * 4 + tt
                        tb, tsub = ttile // 4, ttile % 4
                        ps = ppool.tile([128, 512], f32, tag="ps")
                        for dc in range(8):
                            nc.tensor.matmul(
                                ps[:],
                                xch[(dc, tb)][:, tsub * 128 : (tsub + 1) * 128],
                                wv_sb[:, dc, :],
                                start=(dc == 0),
                                stop=(dc == 7),
                            )
                        nc.scalar.copy(v_sb[:, ttile, :, 0:64], ps[:])
                    if ot == 0:
                        g0 = gen_prepass_pair(0)
                        while next(g0, "end") != "end":
                            pass

            # ============== Phase 2: attention mains ==============
            with tc.tile_pool(name="av_ps", bufs=2, space="PSUM") as avpool:
                th_map = {}

                def gen_main_pair(p):
                    qp, kp = qk_tiles.pop(p)
                    mha = mhat_tiles.pop(p)
                    hA, hB = 2 * p, 2 * p + 1
                    thA = tpool.tile([128, 1024], f16, tag="th", name=f"thA{p}")
                    thB = tpool.tile([128, 1024], f16, tag="th", name=f"thB{p}")
                    th_map[hA], th_map[hB] = thA, thB
                    for qb in range(4):
                        avA = avpool.tile([128, 512], f32, tag="av", name="avA")
                        avB = avpool.tile([128, 512], f32, tag="av", name="avB")
                        nkb = 4 * (qb + 1)
                        pend_av = None
                        for kb in range(nkb):
                            j = kb - 4 * qb
                            t0 = 128 * j if j > 0 else 0
                            diag = j >= 0
                            # fused [A | B] score tile for this k-tile
                            sg = scpool.tile([128, 1024], f32, tag="sc", name="sg")
                            nc.tensor.matmul(
                                sg[:, t0:512],
                                kp[0:64, kb * 128 : (kb + 1) * 128],
                                qp[0:64, qb * 512 + t0 : (qb + 1) * 512],
                                start=True,
                                stop=False,
                                skip_group_check=True,
                                tile_position=(0, 0),
                            )
                            nc.tensor.matmul(
                                sg[:, 512 + t0 : 1024],
                                kp[64:128, kb * 128 : (kb + 1) * 128],
                                qp[64:128, qb * 512 + t0 : (qb + 1) * 512],
                                start=True,
                                stop=False,
                                skip_group_check=True,
                                tile_position=(64, 0),
                            )
                            # -m rank-1 updates on 32-row strips (A/B packed;
                            # kb parity alternates strips so consecutive
                            # k-tiles can overlap too)
                            rA = 32 * (kb % 2)
                            rB = 64 + 32 * (kb % 2)
                            nc.tensor.matmul(
                                sg[:, t0:512],
                                ones_t[rA : rA + 1, :],
                                mha[rA : rA + 1, qb * 512 + t0 : (qb + 1) * 512],
                                start=False,
                                stop=not diag,
                                skip_group_check=True,
                                tile_position=(rA, 0),
                            )
                            nc.tensor.matmul(
                                sg[:, 512 + t0 : 1024],
                                ones_t[rB : rB + 1, :],
                                mha[rB : rB + 1, qb * 512 + t0 : (qb + 1) * 512],
                                start=False,
                                stop=not diag,
                                skip_group_check=True,
                                tile_position=(rB, 0),
                            )
                            # diagonal causal masks
                            if diag:
                                nc.tensor.matmul(
                                    sg[:, t0 : t0 + 128],
                                    negI_t[:],
                                    vmaskD_t[:],
                                    start=False,
                                    stop=True,
                                    skip_group_check=True,
                                )
                                nc.tensor.matmul(
                                    sg[:, 512 + t0 : 512 + t0 + 128],
                                    negI_t[:],
                                    vmaskD_t[:],
                                    start=False,
                                    stop=True,
                                    skip_group_check=True,
                                )
                            # exp -> fp16 w~ ([A | B]); one call covering
                            # [t0:1024] — the stale middle [512,512+t0) of
                            # diag tiles is exp'd but never read downstream
                            wsb = wpoolx.tile([128, 1024], f16, tag="wsb", name="wsb")
                            nc.scalar.activation(wsb[:, t0:1024], sg[:, t0:1024], Exp)

                            # AV software-pipelined by one k-tile: emit the
                            # PREVIOUS kb's AV (its exp surely done) so the PE
                            # never idles waiting on this kb's exp
                            def emit_av(wsb_p, kb_p, t0_p):
                                nc.tensor.matmul(
                                    avA[0:65, t0_p:512],
                                    v_sb[:, kb_p, hA, :],
                                    wsb_p[:, t0_p:512],
                                    start=(kb_p == 0),
                                    stop=(kb_p == nkb - 1),
                                    skip_group_check=True,
                                )
                                nc.tensor.matmul(
                                    avB[0:65, t0_p:512],
                                    v_sb[:, kb_p, hB, :],
                                    wsb_p[:, 512 + t0_p : 1024],
                                    start=(kb_p == 0),
                                    stop=(kb_p == nkb - 1),
                                    skip_group_check=True,
                                )

                            if pend_av is not None:
                                emit_av(*pend_av)
                            pend_av = (wsb, kb, t0)
                            yield
                        if pend_av is not None:
                            emit_av(*pend_av)
                            pend_av = None
                        # drain AV psum; normalize; scatter (reshape bug)
                        for hh, (av, th) in enumerate(((avA, thA), (avB, thB))):
                            avs = smpool.tile([65, 512], f32, tag="avs", name="avs")
                            if hh == 0:
                                nc.scalar.copy(avs[:], av[0:65, :])
                            else:
                                nc.vector.tensor_copy(avs[:], av[0:65, :])
                            # denominator row -> partition-major for cheap DVE
                            # reciprocal, then back to a row for the broadcast
                            rpm = npool.tile([128, 4], f32, tag="rpm", name="rpm")
                            nc.sync.dma_start(rpm[:], avs[64:65, :])
                            rpc = npool.tile([128, 4], f32, tag="rpc", name="rpc")
                            nc.vector.reciprocal(rpc[:], rpm[:])
                            # broadcast to 64 partitions via a DRAM-bounce DMA
                            # (keeps GPSIMD on a single ucode lib: tensor_mul)
                            di = (p * 4 + qb) * 2 + hh
                            nc.sync.dma_start(drow[di : di + 1, :], rpc[:])
                            bc = npool.tile([64, 512], f32, tag="bc", name="bc")
                            nc.sync.dma_start(
                                bc[:], drow[di : di + 1, :].to_broadcast([64, 512])
                            )
                            for jpar in range(2):
                                src = avs[0:64, jpar:512:2].rearrange(
                                    "p (i jj) -> p jj i", jj=8
                                )
                                scl = bc[:, jpar:512:2].rearrange(
                                    "p (i jj) -> p jj i", jj=8
                                )
                                dst = th[
                                    jpar * 64 : jpar * 64 + 64, :
                                ].rearrange("p (jj i4) -> p jj i4", jj=8)[
                                    :, :, qb * 32 : (qb + 1) * 32
                                ]
                                nc.gpsimd.tensor_mul(dst, src, scl)
                            yield

                def gen_wo(h):
                    # output projection, deferred to overlap the next pair's
                    # attention units.  fo lives in the score pool (NOT the
                    # AV pool) so it never blocks the next qb's accumulators.
                    th = th_map.pop(h)
                    for cb in range(2):
                        fo = scpool.tile([128, 512], f32, tag="sc", name="fo")
                        for jj in range(8):
                            nc.tensor.matmul(
                                fo[:],
                                th[:, jj * 128 : (jj + 1) * 128],
                                wo_sb[:, jj, cb * 512 : (cb + 1) * 512],
                                start=(jj == 0),
                                stop=(jj == 7),
                            )
                            if jj == 3:
                                yield
                        fs = fpool.tile([128, 512], f32, tag="fs", name="fs")
                        nc.vector.tensor_copy(fs[:], fo[:])
                        nc.sync.dma_start(
                            out[h * 128 : (h + 1) * 128, cb * 512 : (cb + 1) * 512],
                            fs[:],
                        )
                        yield

                # interleave: main(p) units with prepass(p+1) chunks and
                # Wo(p-1) chunks so all engines stay fed
                pending_wo = []
                for p in range(NP):
                    gm = gen_main_pair(p)
                    gp = gen_prepass_pair(p + 1) if p + 1 < NP else None
                    done_m = False
                    done_p = gp is None
                    step = 0
                    while not done_m or not done_p:
                        # wo first: its MMs are always-ready PE filler that
                        # plugs micro-gaps (keeps HAM at full clock)
                        if pending_wo:
                            if next(pending_wo[0], "end") == "end":
                                pending_wo.pop(0)
                        if not done_m:
                            done_m = next(gm, "end") == "end"
                            if done_m:
                                # release wo immediately — don't serialize it
                                # behind the prepass tail
                                pending_wo.append(gen_wo(2 * p))
                                pending_wo.append(gen_wo(2 * p + 1))
                        if not done_p:
                            done_p = next(gp, "end") == "end"
                        # prepass has ~1.5x the units of main: pump extra so
                        # the pair-boundary barrier (mhat) has no tail
                        if not done_p and step % 2 == 0:
                            done_p = next(gp, "end") == "end"
                        step += 1
                for g in pending_wo:
                    while next(g, "end") != "end":
                        pass
    nc.compile()
    return nc


def _consts():
    p = np.arange(128)[:, None]
    c = np.arange(128)[None, :]
    vmaskD = (c < p).astype(ml_dtypes.bfloat16)
    vmaskP = (c > p).astype(ml_dtypes.bfloat16)
    negI = (NEG * np.eye(128)).astype(ml_dtypes.bfloat16)
    return vmaskD, vmaskP, negI


def kernel(x, Wq, Wk, Wv, Wo):
    x = np.asarray(x, dtype=np.float32)
    Wq = np.asarray(Wq, dtype=np.float32)
    Wk = np.asarray(Wk, dtype=np.float32)
    Wv = np.asarray(Wv, dtype=np.float32)
    Wo = np.asarray(Wo, dtype=np.float32)

    if "nc" not in _CACHE:
        _CACHE["nc"] = _build()
    nc = _CACHE["nc"]

    from concourse.bass_utils import run_bass_kernel_spmd

    vmaskD, vmaskP, negI = _consts()
    woT = np.ascontiguousarray(Wo.T).astype(np.float16)
    in_maps = []
    for c in range(8):
        b, g = c // 2, c % 2
        sl = slice(512 * g, 512 * (g + 1))
        in_maps.append(
            {
                "xT": np.ascontiguousarray(x[b].T).astype(np.float16),
                "wqT": np.ascontiguousarray(Wq[sl, :].T).astype(np.float16),
                "wkT": np.ascontiguousarray(Wk[sl, :].T).astype(np.float16),
                "wvT": np.ascontiguousarray(Wv[sl, :].T).astype(np.float16),
                "woT": woT,
                "vmaskD": vmaskD,
                "vmaskP": vmaskP,
                "negI": negI,
            }
        )
    res = run_bass_kernel_spmd(nc, in_maps, list(range(8)))
    _CACHE["last_result"] = res
    y = np.empty((B, T, D), dtype=np.float32)
    for c in range(8):
        b, g = c // 2, c % 2
        y[b, 1024 * g : 1024 * (g + 1), :] = res.results[c]["out"]
    return y


# revision 28
# speedup vs baseline: 1.0418x; 1.0325x over previous
"""Multi-head causal attention (with faithful reference bugs) on 8 TRN2 cores.

Reference semantics (B=4, T=2048, D=1024, H=16, hd=64):
    q = (x @ Wq.T) viewed (B,T,H,hd) -> (B,H,T,hd); same k, v
    scores = (q @ k.T) * sqrt(D)            # "bug": / D**-0.5
    causal mask, softmax
    out = attn @ v                          # (B,H,T,hd)
    att = out.reshape(B, T, H*hd)           # "bug": no transpose back
    y = att @ Wo.T

Because of the reshape bug, output rows group by head: rows
[128h, 128h+128) of y[b] depend only on head h.  Sharding: 8 cores =
(batch b, head-group g); each core computes y[b, 1024g:1024g+1024, :]
independently (no collectives).

v3 design: heads processed in PAIRS packed onto the PE array.
  - q/k live in paired tiles [128, T]: rows 0-63 = head 2p, 64-127 =
    head 2p+1 (fp16, q pre-scaled by 32).  No DRAM spill.
  - each A/B pair of score matmuls (K=64) targets ONE fused [128,1024]
    PSUM tile ([A | B] halves) at tile_position (0,0)/(64,0): both MMs
    become ready on the same buffer event and run CONCURRENTLY on the
    two halves of the array.
  - prepass (row max): packed q.k chunks + ONE fused DVE reduce per
    chunk ([128, 2, cw] -> [128, 2]) so the tile frees on one event.
  - main: packed q.k + the -m subtraction as K=1 rank-1 matmuls on
    32-row strips + -1e9*I causal mask matmuls; exp -> fp16 w~;
    AV via v^T (ones column emits the denominator row).
  - normalization: denominator row bounced [1,512]->[128,4] via
    SBUF->SBUF DMA so DVE reciprocal runs partition-major; GPSIMD
    broadcast; reshape-bug scatter-multiply on GPSIMD (keeps DVE free
    for the prepass reductions).
  - PSUM: shared score pool (3 x [128,1024] = 6 banks) for prepass
    chunks AND main s~ tiles + 2 AV banks = 8.
"""

import numpy as np
import ml_dtypes

B, T, D, H = 4, 2048, 1024, 16
HD = D // H  # 64
HL = H // 2  # heads per core = 8
NP = HL // 2  # head pairs per core = 4
SCALE = float(np.sqrt(D))  # 32.0
NEG = -1.0e9

_CACHE = {}


def _build():
    import concourse.bacc as bacc
    import concourse.mybir as mybir
    import concourse.tile as tile
    from concourse.masks import make_identity

    dt = mybir.dt
    f32, f16, bf16 = dt.float32, dt.float16, dt.bfloat16
    Exp = mybir.ActivationFunctionType.Exp
    AX = mybir.AxisListType.X

    nc = bacc.Bacc("TRN2", target_bir_lowering=False, debug=False, num_devices=8)

    # ---- DRAM I/O ----
    xT = nc.dram_tensor("xT", [D, T], f16, kind="ExternalInput")  # x[b].T
    wqT = nc.dram_tensor("wqT", [D, 512], f16, kind="ExternalInput")  # Wq[g].T
    wkT = nc.dram_tensor("wkT", [D, 512], f16, kind="ExternalInput")
    wvT = nc.dram_tensor("wvT", [D, 512], f16, kind="ExternalInput")
    woT = nc.dram_tensor("woT", [D, D], f16, kind="ExternalInput")  # Wo.T fp16
    # 0/1 step masks (c<p), (c>p) and -1e9*I for PE-side causal masking
    vmaskD = nc.dram_tensor("vmaskD", [128, 128], bf16, kind="ExternalInput")
    vmaskP = nc.dram_tensor("vmaskP", [128, 128], bf16, kind="ExternalInput")
    negI = nc.dram_tensor("negI", [128, 128], bf16, kind="ExternalInput")
    out = nc.dram_tensor("out", [1024, D], f32, kind="ExternalOutput")
    # scratch rows for the DMA-broadcast of 1/denominator (one per qb/head)
    drow = nc.dram_tensor("drow", [32, 512], f32)

    with tile.TileContext(nc) as tc:
        with (
            tc.tile_pool(name="const", bufs=1) as cpool,
            tc.tile_pool(name="vres", bufs=1) as vpool,
            tc.tile_pool(name="qk", bufs=4) as qkpool,
            tc.tile_pool(name="stat", bufs=4) as mpool,
            tc.tile_pool(name="mhatp", bufs=2) as mhpool,
            tc.tile_pool(name="wexp", bufs=8) as wpoolx,
            tc.tile_pool(name="tt", bufs=6) as tpool,
            tc.tile_pool(name="sm", bufs=6) as smpool,
            tc.tile_pool(name="nrm", bufs=4) as npool,
            tc.tile_pool(name="fsp", bufs=3) as fpool,
            # fused A|B score tiles: prepass chunks AND main s~ tiles
            tc.tile_pool(name="score_ps", bufs=3, space="PSUM") as scpool,
        ):
            # ---- constants / resident tensors ----
            vmaskD_t = cpool.tile([128, 128], bf16)
            nc.sync.dma_start(vmaskD_t[:], vmaskD[:])
            vmaskP_t = cpool.tile([128, 128], bf16)
            nc.sync.dma_start(vmaskP_t[:], vmaskP[:])
            negI_t = cpool.tile([128, 128], bf16)
            nc.sync.dma_start(negI_t[:], negI[:])
            ident = cpool.tile([128, 128], f32)
            make_identity(nc, ident[:])
            ones_t = cpool.tile([128, 128], f16)
            nc.gpsimd.memset(ones_t[:], 1.0)
            wo_sb = cpool.tile([128, 8, 1024], f16)
            nc.sync.dma_start(wo_sb[:], woT.rearrange("(a p) m -> p a m", p=128))
            # v resident: [p, ttile, head, 65] fp16, col 64 = ones
            v_sb = vpool.tile([128, 16, HL, 65], f16)
            nc.gpsimd.memset(v_sb[:, :, :, 64:65], 1.0)

            qk_tiles = {}  # pair -> (qpair, kpair), rows 0-63 = head 2p
            mhat_tiles = {}  # pair -> [128, T] f16, -m at rows {0,32}=A {64,96}=B

            def gen_prepass_pair(p):
                qp, kp = qk_tiles[p]
                mhA = mpool.tile([128, 16], f32, tag="mh", name=f"mhA{p}")
                mhB = mpool.tile([128, 16], f32, tag="mh", name=f"mhB{p}")
                for qi in range(16):
                    kext = 128 * (qi + 1)
                    nchk = (kext + 511) // 512
                    cm = mpool.tile([128, 8], f32, tag="cmx", name="cm")
                    for ch in range(nchk):
                        c0 = ch * 512
                        cw = min(512, kext - c0)
                        last = ch == nchk - 1
                        pr = scpool.tile([128, 1024], f32, tag="sc", name="pr")
                        nc.tensor.matmul(
                            pr[:, 0:cw],
                            qp[0:64, qi * 128 : (qi + 1) * 128],
                            kp[0:64, c0 : c0 + cw],
                            start=True,
                            stop=not last,
                            skip_group_check=True,
                            tile_position=(0, 0),
                        )
                        nc.tensor.matmul(
                            pr[:, 512 : 512 + cw],
                            qp[64:128, qi * 128 : (qi + 1) * 128],
                            kp[64:128, c0 : c0 + cw],
                            start=True,
                            stop=not last,
                            skip_group_check=True,
                            tile_position=(64, 0),
                        )
                        if last:
                            # diagonal causal mask accumulated on the PE
                            nc.tensor.matmul(
                                pr[:, cw - 128 : cw],
                                negI_t[:],
                                vmaskP_t[:],
                                start=False,
                                stop=True,
                                skip_group_check=True,
                            )
                            nc.tensor.matmul(
                                pr[:, 384 + cw : 512 + cw],
                                negI_t[:],
                                vmaskP_t[:],
                                start=False,
                                stop=True,
                                skip_group_check=True,
                            )
                        # fused per-head chunk max: [128, 2, cw] -> [128, 2]
                        nc.vector.reduce_max(
                            cm[:, 2 * ch : 2 * ch + 2],
                            pr[:].rearrange("q (h f) -> q h f", h=2)[:, :, 0:cw],
                            axis=AX,
                        )
                        yield
                    for hh, mh in ((0, mhA), (1, mhB)):
                        if nchk == 1:
                            nc.vector.tensor_scalar_mul(
                                mh[:, qi : qi + 1], cm[:, hh : hh + 1], -1.0
                            )
                        else:
                            nc.vector.reduce_max(
                                mh[:, qi : qi + 1],
                                cm[:, hh : 2 * nchk : 2],
                                axis=AX,
                                negate=True,
                            )
                # -m transposed to q-major rows, replicated at 32-row strips
                mha = mhpool.tile([128, T], f16, tag="mhat", name=f"mhat{p}")
                mhat_tiles[p] = mha
                for mh, rows in ((mhA, (0, 32)), (mhB, (64, 96))):
                    mt_ps = scpool.tile([16, 128], f32, tag="sc", name="mt_ps")
                    nc.tensor.transpose(mt_ps[:], mh[:], ident[:])
                    mts = mpool.tile([16, 128], f16, tag="mts", name="mts")
                    nc.vector.tensor_copy(mts[:], mt_ps[:])
                    for r in rows:
                        nc.sync.dma_start(mha[r : r + 1, :], mts[:])
                    yield

            # ================= Phase 1: projections =================
            with (
                tc.tile_pool(name="wgt", bufs=1) as wpool,
                tc.tile_pool(name="xch", bufs=32) as xpool,
                tc.tile_pool(name="proj_ps", bufs=2, space="PSUM") as ppool,
            ):
                wq_sb = wpool.tile([128, 8, 512], f16, tag="w")
                wk_sb = wpool.tile([128, 8, 512], f16, tag="wk")
                wv_sb = wpool.tile([128, 8, 512], f16, tag="wv")
                # issue weight loads from the ACT queue so they don't
                # serialize behind the x-chunk loads at startup; split each
                # into dc-halves so the first projection MMs start sooner
                wqR = wqT.rearrange("(a p) m -> p a m", p=128)
                wkR = wkT.rearrange("(a p) m -> p a m", p=128)
                wvR = wvT.rearrange("(a p) m -> p a m", p=128)
                for w_dst, w_src in ((wq_sb, wqR), (wk_sb, wkR), (wv_sb, wvR)):
                    nc.scalar.dma_start(w_dst[:, 0:2], w_src[:, 0:2])
                for w_dst, w_src in ((wq_sb, wqR), (wk_sb, wkR), (wv_sb, wvR)):
                    nc.scalar.dma_start(w_dst[:, 2:8], w_src[:, 2:8])

                xTr = xT.rearrange("(a p) m -> p a m", p=128)
                xch = {}
                for tb in range(4):
                    for dc in range(8):
                        xc = xpool.tile([128, 512], f16, tag="x", name=f"x{dc}_{tb}")
                        # split across two DMA queues to halve the startup
                        # critical path
                        eng = nc.gpsimd if dc % 2 == 0 else nc.sync
                        eng.dma_start(xc[:], xTr[:, dc, tb * 512 : (tb + 1) * 512])
                        xch[(dc, tb)] = xc

                for p in range(NP):
                    qpt = qkpool.tile([128, T], f16, tag="qp", name=f"qp{p}")
                    kpt = qkpool.tile([128, T], f16, tag="kp", name=f"kp{p}")
                    qk_tiles[p] = (qpt, kpt)

                for ot in range(4):
                    for wi, (w_sb, scl) in enumerate(
                        ((wq_sb, SCALE), (wk_sb, 1.0))
                    ):
                        for tb in range(4):
                            ps = ppool.tile([128, 512], f32, tag="ps")
                            for dc in range(8):
                                nc.tensor.matmul(
                                    ps[:],
                                    w_sb[:, dc, ot * 128 : (ot + 1) * 128],
                                    xch[(dc, tb)][:],
                                    start=(dc == 0),
                                    stop=(dc == 7),
                                )
                            dst = qk_tiles[ot][wi][:, tb * 512 : (tb + 1) * 512]
                            if scl == 1.0:
                                nc.scalar.copy(dst, ps[:])
                            else:
                                nc.scalar.mul(dst, ps[:], scl)
                    # v projection for this otile's t-range
                    for tt in range(4):
                        ttile = ot * 4 + tt
                        tb, tsub = ttile // 4, ttile % 4
                        ps = ppool.tile([128, 512], f32, tag="ps")
                        for dc in range(8):
                            nc.tensor.matmul(
                                ps[:],
                                xch[(dc, tb)][:, tsub * 128 : (tsub + 1) * 128],
                                wv_sb[:, dc, :],
                                start=(dc == 0),
                                stop=(dc == 7),
                            )
                        nc.scalar.copy(v_sb[:, ttile, :, 0:64], ps[:])
                    if ot == 0:
                        g0 = gen_prepass_pair(0)
                        while next(g0, "end") != "end":
                            pass

            # ============== Phase 2: attention mains ==============
            with tc.tile_pool(name="av_ps", bufs=2, space="PSUM") as avpool:
                th_map = {}

                def gen_main_pair(p):
                    qp, kp = qk_tiles.pop(p)
                    mha = mhat_tiles.pop(p)
                    hA, hB = 2 * p, 2 * p + 1
                    thA = tpool.tile([128, 1024], f16, tag="th", name=f"thA{p}")
                    thB = tpool.tile([128, 1024], f16, tag="th", name=f"thB{p}")
                    th_map[hA], th_map[hB] = thA, thB
                    for qb in range(4):
                        avA = avpool.tile([128, 512], f32, tag="av", name="avA")
                        avB = avpool.tile([128, 512], f32, tag="av", name="avB")
                        nkb = 4 * (qb + 1)
                        pend_av = None
                        for kb in range(nkb):
                            j = kb - 4 * qb
                            t0 = 128 * j if j > 0 else 0
                            diag = j >= 0
                            # fused [A | B] score tile for this k-tile
                            sg = scpool.tile([128, 1024], f32, tag="sc", name="sg")
                            nc.tensor.matmul(
                                sg[:, t0:512],
                                kp[0:64, kb * 128 : (kb + 1) * 128],
                                qp[0:64, qb * 512 + t0 : (qb + 1) * 512],
                                start=True,
                                stop=False,
                                skip_group_check=True,
                                tile_position=(0, 0),
                            )
                            nc.tensor.matmul(
                                sg[:, 512 + t0 : 1024],
                                kp[64:128, kb * 128 : (kb + 1) * 128],
                                qp[64:128, qb * 512 + t0 : (qb + 1) * 512],
                                start=True,
                                stop=False,
                                skip_group_check=True,
                                tile_position=(64, 0),
                            )
                            # -m rank-1 updates on 32-row strips (A/B packed;
                            # kb parity alternates strips so consecutive
                            # k-tiles can overlap too)
                            rA = 32 * (kb % 2)
                            rB = 64 + 32 * (kb % 2)
                            nc.tensor.matmul(
                                sg[:, t0:512],
                                ones_t[rA : rA + 1, :],
                                mha[rA : rA + 1, qb * 512 + t0 : (qb + 1) * 512],
                                start=False,
                                stop=not diag,
                                skip_group_check=True,
                                tile_position=(rA, 0),
                            )
                            nc.tensor.matmul(
                                sg[:, 512 + t0 : 1024],
                                ones_t[rB : rB + 1, :],
                                mha[rB : rB + 1, qb * 512 + t0 : (qb + 1) * 512],
                                start=False,
                                stop=not diag,
                                skip_group_check=True,
                                tile_position=(rB, 0),
                            )
                            # diagonal causal masks
                            if diag:
                                nc.tensor.matmul(
                                    sg[:, t0 : t0 + 128],
                                    negI_t[:],
                                    vmaskD_t[:],
                                    start=False,
                                    stop=True,
                                    skip_group_check=True,
                                )
                                nc.tensor.matmul(
                                    sg[:, 512 + t0 : 512 + t0 + 128],
                                    negI_t[:],
                                    vmaskD_t[:],
                                    start=False,
                                    stop=True,
                                    skip_group_check=True,
                                )
                            # exp -> fp16 w~ ([A | B]); one call covering
                            # [t0:1024] — the stale middle [512,512+t0) of
                            # diag tiles is exp'd but never read downstream
                            wsb = wpoolx.tile([128, 1024], f16, tag="wsb", name="wsb")
                            nc.scalar.activation(wsb[:, t0:1024], sg[:, t0:1024], Exp)

                            # AV software-pipelined by one k-tile: emit the
                            # PREVIOUS kb's AV (its exp surely done) so the PE
                            # never idles waiting on this kb's exp
                            def emit_av(wsb_p, kb_p, t0_p):
                                nc.tensor.matmul(
                                    avA[0:65, t0_p:512],
                                    v_sb[:, kb_p, hA, :],
                                    wsb_p[:, t0_p:512],
                                    start=(kb_p == 0),
                                    stop=(kb_p == nkb - 1),
                                    skip_group_check=True,
                                )
                                nc.tensor.matmul(
                                    avB[0:65, t0_p:512],
                                    v_sb[:, kb_p, hB, :],
                                    wsb_p[:, 512 + t0_p : 1024],
                                    start=(kb_p == 0),
                                    stop=(kb_p == nkb - 1),
                                    skip_group_check=True,
                                )

                            if pend_av is not None:
                                emit_av(*pend_av)
                            pend_av = (wsb, kb, t0)
                            yield
                        if pend_av is not None:
                            emit_av(*pend_av)
                            pend_av = None
                        # drain AV psum; normalize; scatter (reshape bug)
                        for hh, (av, th) in enumerate(((avA, thA), (avB, thB))):
                            avs = smpool.tile([65, 512], f32, tag="avs", name="avs")
                            if hh == 0:
                                nc.scalar.copy(avs[:], av[0:65, :])
                            else:
                                nc.vector.tensor_copy(avs[:], av[0:65, :])
                            # denominator row -> partition-major for cheap DVE
                            # reciprocal, then back to a row for the broadcast
                            rpm = npool.tile([128, 4], f32, tag="rpm", name="rpm")
                            nc.sync.dma_start(rpm[:], avs[64:65, :])
                            rpc = npool.tile([128, 4], f32, tag="rpc", name="rpc")
                            nc.vector.reciprocal(rpc[:], rpm[:])
                            # broadcast to 64 partitions via a DRAM-bounce DMA
                            # (keeps GPSIMD on a single ucode lib: tensor_mul)
                            di = (p * 4 + qb) * 2 + hh
                            nc.sync.dma_start(drow[di : di + 1, :], rpc[:])
                            bc = npool.tile([64, 512], f32, tag="bc", name="bc")
                            nc.sync.dma_start(
                                bc[:], drow[di : di + 1, :].to_broadcast([64, 512])
                            )
                            for jpar in range(2):
                                src = avs[0:64, jpar:512:2].rearrange(
                                    "p (i jj) -> p jj i", jj=8
                                )
                                scl = bc[:, jpar:512:2].rearrange(
                                    "p (i jj) -> p jj i", jj=8
                                )
                                dst = th[
                                    jpar * 64 : jpar * 64 + 64, :
                                ].rearrange("p (jj i4) -> p jj i4", jj=8)[
                                    :, :, qb * 32 : (qb + 1) * 32
                                ]
                                nc.gpsimd.tensor_mul(dst, src, scl)
                            yield

                def gen_wo(h):
                    # output projection, deferred to overlap the next pair's
                    # attention units.  fo lives in the score pool (NOT the
                    # AV pool) so it never blocks the next qb's accumulators.
                    th = th_map.pop(h)
                    for cb in range(2):
                        fo = scpool.tile([128, 512], f32, tag="sc", name="fo")
                        for jj in range(8):
                            nc.tensor.matmul(
                                fo[:],
                                th[:, jj * 128 : (jj + 1) * 128],
                                wo_sb[:, jj, cb * 512 : (cb + 1) * 512],
                                start=(jj == 0),
                                stop=(jj == 7),
                            )
                            if jj == 3:
                                yield
                        fs = fpool.tile([128, 512], f32, tag="fs", name="fs")
                        nc.vector.tensor_copy(fs[:], fo[:])
                        nc.sync.dma_start(
                            out[h * 128 : (h + 1) * 128, cb * 512 : (cb + 1) * 512],
                            fs[:],
                        )
                        yield

                # interleave: main(p) units with prepass(p+1) chunks and
                # Wo(p-1) chunks so all engines stay fed
                pending_wo = []
                for p in range(NP):
                    gm = gen_main_pair(p)
                    gp = gen_prepass_pair(p + 1) if p + 1 < NP else None
                    done_m = False
                    done_p = gp is None
                    step = 0
                    while not done_m or not done_p:
                        # wo first: its MMs are always-ready PE filler that
                        # plugs micro-gaps (keeps HAM at full clock)
                        if pending_wo:
                            if next(pending_wo[0], "end") == "end":
                                pending_wo.pop(0)
                        if not done_m:
                            done_m = next(gm, "end") == "end"
                            if done_m:
                                # release wo immediately — don't serialize it
                                # behind the prepass tail
                                pending_wo.append(gen_wo(2 * p))
                                pending_wo.append(gen_wo(2 * p + 1))
                        if not done_p:
                            done_p = next(gp, "end") == "end"
                        # prepass has ~1.5x the units of main: pump extra so
                        # the pair-boundary barrier (mhat) has no tail
                        if not done_p and step % 2 == 0:
                            done_p = next(gp, "end") == "end"
                        step += 1
                for g in pending_wo:
                    while next(g, "end") != "end":
                        pass
    nc.compile()
    return nc


def _consts():
    p = np.arange(128)[:, None]
    c = np.arange(128)[None, :]
    vmaskD = (c < p).astype(ml_dtypes.bfloat16)
    vmaskP = (c > p).astype(ml_dtypes.bfloat16)
    negI = (NEG * np.eye(128)).astype(ml_dtypes.bfloat16)
    return vmaskD, vmaskP, negI


def kernel(x, Wq, Wk, Wv, Wo):
    x = np.asarray(x, dtype=np.float32)
    Wq = np.asarray(Wq, dtype=np.float32)
    Wk = np.asarray(Wk, dtype=np.float32)
    Wv = np.asarray(Wv, dtype=np.float32)
    Wo = np.asarray(Wo, dtype=np.float32)

    if "nc" not in _CACHE:
        _CACHE["nc"] = _build()
    nc = _CACHE["nc"]

    from concourse.bass_utils import run_bass_kernel_spmd

    vmaskD, vmaskP, negI = _consts()
    woT = np.ascontiguousarray(Wo.T).astype(np.float16)
    in_maps = []
    for c in range(8):
        b, g = c // 2, c % 2
        sl = slice(512 * g, 512 * (g + 1))
        in_maps.append(
            {
                "xT": np.ascontiguousarray(x[b].T).astype(np.float16),
                "wqT": np.ascontiguousarray(Wq[sl, :].T).astype(np.float16),
                "wkT": np.ascontiguousarray(Wk[sl, :].T).astype(np.float16),
                "wvT": np.ascontiguousarray(Wv[sl, :].T).astype(np.float16),
                "woT": woT,
                "vmaskD": vmaskD,
                "vmaskP": vmaskP,
                "negI": negI,
            }
        )
    res = run_bass_kernel_spmd(nc, in_maps, list(range(8)))
    _CACHE["last_result"] = res
    y = np.empty((B, T, D), dtype=np.float32)
    for c in range(8):
        b, g = c // 2, c % 2
        y[b, 1024 * g : 1024 * (g + 1), :] = res.results[c]["out"]
    return y


# revision 29
# speedup vs baseline: 1.0720x; 1.0290x over previous
"""Multi-head causal attention (with faithful reference bugs) on 8 TRN2 cores.

Reference semantics (B=4, T=2048, D=1024, H=16, hd=64):
    q = (x @ Wq.T) viewed (B,T,H,hd) -> (B,H,T,hd); same k, v
    scores = (q @ k.T) * sqrt(D)            # "bug": / D**-0.5
    causal mask, softmax
    out = attn @ v                          # (B,H,T,hd)
    att = out.reshape(B, T, H*hd)           # "bug": no transpose back
    y = att @ Wo.T

Because of the reshape bug, output rows group by head: rows
[128h, 128h+128) of y[b] depend only on head h.  Sharding: 8 cores =
(batch b, head-group g); each core computes y[b, 1024g:1024g+1024, :]
independently (no collectives).

v3 design: heads processed in PAIRS packed onto the PE array.
  - q/k live in paired tiles [128, T]: rows 0-63 = head 2p, 64-127 =
    head 2p+1 (fp16, q pre-scaled by 32).  No DRAM spill.
  - each A/B pair of score matmuls (K=64) targets ONE fused [128,1024]
    PSUM tile ([A | B] halves) at tile_position (0,0)/(64,0): both MMs
    become ready on the same buffer event and run CONCURRENTLY on the
    two halves of the array.
  - prepass (row max): packed q.k chunks + ONE fused DVE reduce per
    chunk ([128, 2, cw] -> [128, 2]) so the tile frees on one event.
  - main: packed q.k + the -m subtraction as K=1 rank-1 matmuls on
    32-row strips + -1e9*I causal mask matmuls; exp -> fp16 w~;
    AV via v^T (ones column emits the denominator row).
  - normalization: denominator row bounced [1,512]->[128,4] via
    SBUF->SBUF DMA so DVE reciprocal runs partition-major; GPSIMD
    broadcast; reshape-bug scatter-multiply on GPSIMD (keeps DVE free
    for the prepass reductions).
  - PSUM: shared score pool (3 x [128,1024] = 6 banks) for prepass
    chunks AND main s~ tiles + 2 AV banks = 8.
"""

import numpy as np
import ml_dtypes

B, T, D, H = 4, 2048, 1024, 16
HD = D // H  # 64
HL = H // 2  # heads per core = 8
NP = HL // 2  # head pairs per core = 4
SCALE = float(np.sqrt(D))  # 32.0
NEG = -1.0e9

_CACHE = {}


def _build():
    import concourse.bacc as bacc
    import concourse.mybir as mybir
    import concourse.tile as tile
    from concourse.masks import make_identity

    dt = mybir.dt
    f32, f16, bf16 = dt.float32, dt.float16, dt.bfloat16
    Exp = mybir.ActivationFunctionType.Exp
    AX = mybir.AxisListType.X

    nc = bacc.Bacc("TRN2", target_bir_lowering=False, debug=False, num_devices=8)

    # ---- DRAM I/O ----
    xT = nc.dram_tensor("xT", [D, T], f16, kind="ExternalInput")  # x[b].T
    wqT = nc.dram_tensor("wqT", [D, 512], f16, kind="ExternalInput")  # Wq[g].T
    wkT = nc.dram_tensor("wkT", [D, 512], f16, kind="ExternalInput")
    wvT = nc.dram_tensor("wvT", [D, 512], f16, kind="ExternalInput")
    woT = nc.dram_tensor("woT", [D, D], f16, kind="ExternalInput")  # Wo.T fp16
    # 0/1 step masks (c<p), (c>p) and -1e9*I for PE-side causal masking
    vmaskD = nc.dram_tensor("vmaskD", [128, 128], bf16, kind="ExternalInput")
    vmaskP = nc.dram_tensor("vmaskP", [128, 128], bf16, kind="ExternalInput")
    negI = nc.dram_tensor("negI", [128, 128], bf16, kind="ExternalInput")
    out = nc.dram_tensor("out", [1024, D], f32, kind="ExternalOutput")
    # scratch rows for the DMA-broadcast of 1/denominator (one per qb/head)
    drow = nc.dram_tensor("drow", [32, 512], f32)

    with tile.TileContext(nc) as tc:
        with (
            tc.tile_pool(name="const", bufs=1) as cpool,
            tc.tile_pool(name="vres", bufs=1) as vpool,
            tc.tile_pool(name="qk", bufs=4) as qkpool,
            tc.tile_pool(name="stat", bufs=4) as mpool,
            tc.tile_pool(name="mhatp", bufs=2) as mhpool,
            tc.tile_pool(name="wexp", bufs=8) as wpoolx,
            tc.tile_pool(name="tt", bufs=6) as tpool,
            tc.tile_pool(name="sm", bufs=6) as smpool,
            tc.tile_pool(name="nrm", bufs=4) as npool,
            tc.tile_pool(name="fsp", bufs=3) as fpool,
            # fused A|B score tiles: prepass chunks AND main s~ tiles
            tc.tile_pool(name="score_ps", bufs=3, space="PSUM") as scpool,
        ):
            # ---- constants / resident tensors ----
            vmaskD_t = cpool.tile([128, 128], bf16)
            nc.sync.dma_start(vmaskD_t[:], vmaskD[:])
            vmaskP_t = cpool.tile([128, 128], bf16)
            nc.sync.dma_start(vmaskP_t[:], vmaskP[:])
            negI_t = cpool.tile([128, 128], bf16)
            nc.sync.dma_start(negI_t[:], negI[:])
            ident = cpool.tile([128, 128], f32)
            make_identity(nc, ident[:])
            ones_t = cpool.tile([128, 128], f16)
            nc.gpsimd.memset(ones_t[:], 1.0)
            wo_sb = cpool.tile([128, 8, 1024], f16)
            nc.sync.dma_start(wo_sb[:], woT.rearrange("(a p) m -> p a m", p=128))
            # v resident: [p, ttile, head, 65] fp16, col 64 = ones
            v_sb = vpool.tile([128, 16, HL, 65], f16)
            nc.gpsimd.memset(v_sb[:, :, :, 64:65], 1.0)

            qk_tiles = {}  # pair -> (qpair, kpair), rows 0-63 = head 2p
            mhat_tiles = {}  # pair -> [128, T] f16, -m at rows {0,32}=A {64,96}=B

            def gen_prepass_pair(p):
                qp, kp = qk_tiles[p]
                mhA = mpool.tile([128, 16], f32, tag="mh", name=f"mhA{p}")
                mhB = mpool.tile([128, 16], f32, tag="mh", name=f"mhB{p}")
                for qi in range(16):
                    kext = 128 * (qi + 1)
                    nchk = (kext + 511) // 512
                    cm = mpool.tile([128, 8], f32, tag="cmx", name="cm")
                    for ch in range(nchk):
                        c0 = ch * 512
                        cw = min(512, kext - c0)
                        last = ch == nchk - 1
                        pr = scpool.tile([128, 1024], f32, tag="sc", name="pr")
                        nc.tensor.matmul(
                            pr[:, 0:cw],
                            qp[0:64, qi * 128 : (qi + 1) * 128],
                            kp[0:64, c0 : c0 + cw],
                            start=True,
                            stop=not last,
                            skip_group_check=True,
                            tile_position=(0, 0),
                        )
                        nc.tensor.matmul(
                            pr[:, 512 : 512 + cw],
                            qp[64:128, qi * 128 : (qi + 1) * 128],
                            kp[64:128, c0 : c0 + cw],
                            start=True,
                            stop=not last,
                            skip_group_check=True,
                            tile_position=(64, 0),
                        )
                        if last:
                            # diagonal causal mask accumulated on the PE
                            nc.tensor.matmul(
                                pr[:, cw - 128 : cw],
                                negI_t[:],
                                vmaskP_t[:],
                                start=False,
                                stop=True,
                                skip_group_check=True,
                            )
                            nc.tensor.matmul(
                                pr[:, 384 + cw : 512 + cw],
                                negI_t[:],
                                vmaskP_t[:],
                                start=False,
                                stop=True,
                                skip_group_check=True,
                            )
                        # fused per-head chunk max: [128, 2, cw] -> [128, 2]
                        nc.vector.reduce_max(
                            cm[:, 2 * ch : 2 * ch + 2],
                            pr[:].rearrange("q (h f) -> q h f", h=2)[:, :, 0:cw],
                            axis=AX,
                        )
                        yield
                    for hh, mh in ((0, mhA), (1, mhB)):
                        if nchk == 1:
                            nc.vector.tensor_scalar_mul(
                                mh[:, qi : qi + 1], cm[:, hh : hh + 1], -1.0
                            )
                        else:
                            nc.vector.reduce_max(
                                mh[:, qi : qi + 1],
                                cm[:, hh : 2 * nchk : 2],
                                axis=AX,
                                negate=True,
                            )
                # -m transposed to q-major rows, replicated at 32-row strips
                mha = mhpool.tile([128, T], f16, tag="mhat", name=f"mhat{p}")
                mhat_tiles[p] = mha
                for mh, rows in ((mhA, (0, 32)), (mhB, (64, 96))):
                    mt_ps = scpool.tile([16, 128], f32, tag="sc", name="mt_ps")
                    nc.tensor.transpose(mt_ps[:], mh[:], ident[:])
                    mts = mpool.tile([16, 128], f16, tag="mts", name="mts")
                    nc.vector.tensor_copy(mts[:], mt_ps[:])
                    for r in rows:
                        nc.sync.dma_start(mha[r : r + 1, :], mts[:])
                    yield

            # ================= Phase 1: projections =================
            with (
                tc.tile_pool(name="wgt", bufs=1) as wpool,
                tc.tile_pool(name="xch", bufs=32) as xpool,
                tc.tile_pool(name="proj_ps", bufs=2, space="PSUM") as ppool,
            ):
                wq_sb = wpool.tile([128, 8, 512], f16, tag="w")
                wk_sb = wpool.tile([128, 8, 512], f16, tag="wk")
                wv_sb = wpool.tile([128, 8, 512], f16, tag="wv")
                # issue weight loads from the ACT queue so they don't
                # serialize behind the x-chunk loads at startup; split each
                # into dc-halves so the first projection MMs start sooner
                wqR = wqT.rearrange("(a p) m -> p a m", p=128)
                wkR = wkT.rearrange("(a p) m -> p a m", p=128)
                wvR = wvT.rearrange("(a p) m -> p a m", p=128)
                for w_dst, w_src in ((wq_sb, wqR), (wk_sb, wkR), (wv_sb, wvR)):
                    nc.scalar.dma_start(w_dst[:, 0:2], w_src[:, 0:2])
                for w_dst, w_src in ((wq_sb, wqR), (wk_sb, wkR), (wv_sb, wvR)):
                    nc.scalar.dma_start(w_dst[:, 2:8], w_src[:, 2:8])

                xTr = xT.rearrange("(a p) m -> p a m", p=128)
                xch = {}
                for tb in range(4):
                    for dc in range(8):
                        xc = xpool.tile([128, 512], f16, tag="x", name=f"x{dc}_{tb}")
                        # split across two DMA queues to halve the startup
                        # critical path
                        eng = nc.gpsimd if dc % 2 == 0 else nc.sync
                        eng.dma_start(xc[:], xTr[:, dc, tb * 512 : (tb + 1) * 512])
                        xch[(dc, tb)] = xc

                for p in range(NP):
                    qpt = qkpool.tile([128, T], f16, tag="qp", name=f"qp{p}")
                    kpt = qkpool.tile([128, T], f16, tag="kp", name=f"kp{p}")
                    qk_tiles[p] = (qpt, kpt)

                for ot in range(4):
                    for wi, (w_sb, scl) in enumerate(
                        ((wq_sb, SCALE), (wk_sb, 1.0))
                    ):
                        for tb in range(4):
                            ps = ppool.tile([128, 512], f32, tag="ps")
                            for dc in range(8):
                                nc.tensor.matmul(
                                    ps[:],
                                    w_sb[:, dc, ot * 128 : (ot + 1) * 128],
                                    xch[(dc, tb)][:],
                                    start=(dc == 0),
                                    stop=(dc == 7),
                                )
                            dst = qk_tiles[ot][wi][:, tb * 512 : (tb + 1) * 512]
                            if scl == 1.0:
                                nc.scalar.copy(dst, ps[:])
                            else:
                                nc.scalar.mul(dst, ps[:], scl)
                    # v projection for this otile's t-range
                    for tt in range(4):
                        ttile = ot * 4 + tt
                        tb, tsub = ttile // 4, ttile % 4
                        ps = ppool.tile([128, 512], f32, tag="ps")
                        for dc in range(8):
                            nc.tensor.matmul(
                                ps[:],
                                xch[(dc, tb)][:, tsub * 128 : (tsub + 1) * 128],
                                wv_sb[:, dc, :],
                                start=(dc == 0),
                                stop=(dc == 7),
                            )
                        nc.scalar.copy(v_sb[:, ttile, :, 0:64], ps[:])
                    if ot == 0:
                        g0 = gen_prepass_pair(0)
                        while next(g0, "end") != "end":
                            pass

            # ============== Phase 2: attention mains ==============
            with tc.tile_pool(name="av_ps", bufs=2, space="PSUM") as avpool:
                th_map = {}

                def gen_main_pair(p):
                    qp, kp = qk_tiles.pop(p)
                    mha = mhat_tiles.pop(p)
                    hA, hB = 2 * p, 2 * p + 1
                    thA = tpool.tile([128, 1024], f16, tag="th", name=f"thA{p}")
                    thB = tpool.tile([128, 1024], f16, tag="th", name=f"thB{p}")
                    th_map[hA], th_map[hB] = thA, thB
                    for qb in range(4):
                        avA = avpool.tile([128, 512], f32, tag="av", name="avA")
                        avB = avpool.tile([128, 512], f32, tag="av", name="avB")
                        nkb = 4 * (qb + 1)
                        pend_av = None
                        for kb in range(nkb):
                            j = kb - 4 * qb
                            t0 = 128 * j if j > 0 else 0
                            diag = j >= 0
                            # fused [A | B] score tile for this k-tile
                            sg = scpool.tile([128, 1024], f32, tag="sc", name="sg")
                            nc.tensor.matmul(
                                sg[:, t0:512],
                                kp[0:64, kb * 128 : (kb + 1) * 128],
                                qp[0:64, qb * 512 + t0 : (qb + 1) * 512],
                                start=True,
                                stop=False,
                                skip_group_check=True,
                                tile_position=(0, 0),
                            )
                            nc.tensor.matmul(
                                sg[:, 512 + t0 : 1024],
                                kp[64:128, kb * 128 : (kb + 1) * 128],
                                qp[64:128, qb * 512 + t0 : (qb + 1) * 512],
                                start=True,
                                stop=False,
                                skip_group_check=True,
                                tile_position=(64, 0),
                            )
                            # -m rank-1 updates on 32-row strips (A/B packed;
                            # kb parity alternates strips so consecutive
                            # k-tiles can overlap too)
                            rA = 32 * (kb % 2)
                            rB = 64 + 32 * (kb % 2)
                            nc.tensor.matmul(
                                sg[:, t0:512],
                                ones_t[rA : rA + 1, :],
                                mha[rA : rA + 1, qb * 512 + t0 : (qb + 1) * 512],
                                start=False,
                                stop=not diag,
                                skip_group_check=True,
                                tile_position=(rA, 0),
                            )
                            nc.tensor.matmul(
                                sg[:, 512 + t0 : 1024],
                                ones_t[rB : rB + 1, :],
                                mha[rB : rB + 1, qb * 512 + t0 : (qb + 1) * 512],
                                start=False,
                                stop=not diag,
                                skip_group_check=True,
                                tile_position=(rB, 0),
                            )
                            # diagonal causal masks
                            if diag:
                                nc.tensor.matmul(
                                    sg[:, t0 : t0 + 128],
                                    negI_t[:],
                                    vmaskD_t[:],
                                    start=False,
                                    stop=True,
                                    skip_group_check=True,
                                )
                                nc.tensor.matmul(
                                    sg[:, 512 + t0 : 512 + t0 + 128],
                                    negI_t[:],
                                    vmaskD_t[:],
                                    start=False,
                                    stop=True,
                                    skip_group_check=True,
                                )
                            # exp -> fp16 w~ ([A | B]); one call covering
                            # [t0:1024] — the stale middle [512,512+t0) of
                            # diag tiles is exp'd but never read downstream
                            wsb = wpoolx.tile([128, 1024], f16, tag="wsb", name="wsb")
                            nc.scalar.activation(wsb[:, t0:1024], sg[:, t0:1024], Exp)

                            # AV software-pipelined by one k-tile: emit the
                            # PREVIOUS kb's AV (its exp surely done) so the PE
                            # never idles waiting on this kb's exp
                            def emit_av(wsb_p, kb_p, t0_p):
                                nc.tensor.matmul(
                                    avA[0:65, t0_p:512],
                                    v_sb[:, kb_p, hA, :],
                                    wsb_p[:, t0_p:512],
                                    start=(kb_p == 0),
                                    stop=(kb_p == nkb - 1),
                                    skip_group_check=True,
                                )
                                nc.tensor.matmul(
                                    avB[0:65, t0_p:512],
                                    v_sb[:, kb_p, hB, :],
                                    wsb_p[:, 512 + t0_p : 1024],
                                    start=(kb_p == 0),
                                    stop=(kb_p == nkb - 1),
                                    skip_group_check=True,
                                )

                            if pend_av is not None:
                                emit_av(*pend_av)
                            pend_av = (wsb, kb, t0)
                            yield
                        if pend_av is not None:
                            emit_av(*pend_av)
                            pend_av = None
                        # drain AV psum; normalize; scatter (reshape bug)
                        for hh, (av, th) in enumerate(((avA, thA), (avB, thB))):
                            avs = smpool.tile([65, 512], f32, tag="avs", name="avs")
                            if hh == 0:
                                nc.scalar.copy(avs[:], av[0:65, :])
                            else:
                                nc.vector.tensor_copy(avs[:], av[0:65, :])
                            # denominator row -> partition-major for cheap DVE
                            # reciprocal, then back to a row for the broadcast
                            rpm = npool.tile([128, 4], f32, tag="rpm", name="rpm")
                            nc.sync.dma_start(rpm[:], avs[64:65, :])
                            rpc = npool.tile([128, 4], f32, tag="rpc", name="rpc")
                            nc.vector.reciprocal(rpc[:], rpm[:])
                            # broadcast to 64 partitions via a DRAM-bounce DMA
                            # (keeps GPSIMD on a single ucode lib: tensor_mul)
                            di = (p * 4 + qb) * 2 + hh
                            nc.sync.dma_start(drow[di : di + 1, :], rpc[:])
                            bc = npool.tile([64, 512], f32, tag="bc", name="bc")
                            nc.sync.dma_start(
                                bc[:], drow[di : di + 1, :].to_broadcast([64, 512])
                            )
                            for jpar in range(2):
                                src = avs[0:64, jpar:512:2].rearrange(
                                    "p (i jj) -> p jj i", jj=8
                                )
                                scl = bc[:, jpar:512:2].rearrange(
                                    "p (i jj) -> p jj i", jj=8
                                )
                                dst = th[
                                    jpar * 64 : jpar * 64 + 64, :
                                ].rearrange("p (jj i4) -> p jj i4", jj=8)[
                                    :, :, qb * 32 : (qb + 1) * 32
                                ]
                                nc.gpsimd.tensor_mul(dst, src, scl)
                            yield

                def gen_wo(h):
                    # output projection, deferred to overlap the next pair's
                    # attention units.  fo lives in the score pool (NOT the
                    # AV pool) so it never blocks the next qb's accumulators.
                    th = th_map.pop(h)
                    for cb in range(2):
                        fo = scpool.tile([128, 512], f32, tag="sc", name="fo")
                        for jj in range(8):
                            nc.tensor.matmul(
                                fo[:],
                                th[:, jj * 128 : (jj + 1) * 128],
                                wo_sb[:, jj, cb * 512 : (cb + 1) * 512],
                                start=(jj == 0),
                                stop=(jj == 7),
                            )
                            if jj == 3:
                                yield
                        fs = fpool.tile([128, 512], f32, tag="fs", name="fs")
                        nc.vector.tensor_copy(fs[:], fo[:])
                        nc.sync.dma_start(
                            out[h * 128 : (h + 1) * 128, cb * 512 : (cb + 1) * 512],
                            fs[:],
                        )
                        yield

                # interleave: main(p) units with prepass(p+1) chunks and
                # Wo(p-1) chunks so all engines stay fed
                pending_wo = []
                for p in range(NP):
                    gm = gen_main_pair(p)
                    gp = gen_prepass_pair(p + 1) if p + 1 < NP else None
                    done_m = False
                    done_p = gp is None
                    step = 0
                    while not done_m or not done_p:
                        # wo first: its MMs are always-ready PE filler that
                        # plugs micro-gaps (keeps HAM at full clock)
                        if pending_wo:
                            if next(pending_wo[0], "end") == "end":
                                pending_wo.pop(0)
                        if not done_m:
                            done_m = next(gm, "end") == "end"
                            if done_m:
                                # release wo immediately — don't serialize it
                                # behind the prepass tail
                                pending_wo.append(gen_wo(2 * p))
                                pending_wo.append(gen_wo(2 * p + 1))
                        if not done_p:
                            done_p = next(gp, "end") == "end"
                        # prepass has ~1.5x the units of main: pump it 2x so
                        # the pair-boundary barrier (mhat) has no tail
                        if not done_p:
                            done_p = next(gp, "end") == "end"
                        step += 1
                for g in pending_wo:
                    while next(g, "end") != "end":
                        pass
    nc.compile()
    return nc


def _consts():
    p = np.arange(128)[:, None]
    c = np.arange(128)[None, :]
    vmaskD = (c < p).astype(ml_dtypes.bfloat16)
    vmaskP = (c > p).astype(ml_dtypes.bfloat16)
    negI = (NEG * np.eye(128)).astype(ml_dtypes.bfloat16)
    return vmaskD, vmaskP, negI


def kernel(x, Wq, Wk, Wv, Wo):
    x = np.asarray(x, dtype=np.float32)
    Wq = np.asarray(Wq, dtype=np.float32)
    Wk = np.asarray(Wk, dtype=np.float32)
    Wv = np.asarray(Wv, dtype=np.float32)
    Wo = np.asarray(Wo, dtype=np.float32)

    if "nc" not in _CACHE:
        _CACHE["nc"] = _build()
    nc = _CACHE["nc"]

    from concourse.bass_utils import run_bass_kernel_spmd

    vmaskD, vmaskP, negI = _consts()
    woT = np.ascontiguousarray(Wo.T).astype(np.float16)
    in_maps = []
    for c in range(8):
        b, g = c // 2, c % 2
        sl = slice(512 * g, 512 * (g + 1))
        in_maps.append(
            {
                "xT": np.ascontiguousarray(x[b].T).astype(np.float16),
                "wqT": np.ascontiguousarray(Wq[sl, :].T).astype(np.float16),
                "wkT": np.ascontiguousarray(Wk[sl, :].T).astype(np.float16),
                "wvT": np.ascontiguousarray(Wv[sl, :].T).astype(np.float16),
                "woT": woT,
                "vmaskD": vmaskD,
                "vmaskP": vmaskP,
                "negI": negI,
            }
        )
    res = run_bass_kernel_spmd(nc, in_maps, list(range(8)))
    _CACHE["last_result"] = res
    y = np.empty((B, T, D), dtype=np.float32)
    for c in range(8):
        b, g = c // 2, c % 2
        y[b, 1024 * g : 1024 * (g + 1), :] = res.results[c]["out"]
    return y
